# revision 64
# baseline (speedup 1.0000x reference)
"""Trainium2 Bass kernel for the single-query-attention diffusion decoder.

Full-input contract: kernel(**inputs) -> np.ndarray [B, V].
Data-parallel over batch across 8 NeuronCores (16 rows each).

Math (reference restructured):
    cond  = silu(pe[t] @ Wt1.T + bt1) @ Wt2.T + bt2            [B, D]
    q~    = (query + cond) @ M1,  M1 = Wq.T @ Wk               [B, D]
    s[v]  = q~ . T[v] + x[v]   (+ q~.cond, dropped: softmax shift-invariant)
    w     = softmax(s)
    wsum  = sum_v w[v] T[v] + cond                             [D]
    base  = wsum @ M3 + cond @ Bm + r0                         [2D]
            M3 = Wv.T @ Wp.T @ Wd1[:, :D].T,  Bm = Wd1[:, D:].T
            r0 = bp @ Wd1[:, :D].T + bd1
    p[v]  = sum_j relu(T[v] @ Bm + base)[j] * w2[j] + bd2 + w[v]

The target shard is passed host-transposed [16, D, V] in bf16 so the PE
contraction dim (d) lands on SBUF partitions with no on-device transposes
and half the HBM traffic. Matmuls run in bf16 (PSUM accumulates fp32);
q~ uses a bf16 hi+lo split to keep softmax logits near-fp32 accurate; the
decoder relu/w2 reduction and all final adds stay in fp32.
"""

import os
import sys

for _p in ("/opt/trn_rl_repo", "/opt/trn_rl_repo/concourse"):
    if os.path.isdir(_p) and _p not in sys.path:
        sys.path.append(_p)

import numpy as np
import ml_dtypes

import concourse.bass as bass
import concourse.tile as tile
from concourse import bacc, mybir
from concourse.bass_utils import run_bass_kernel_spmd

F32 = mybir.dt.float32
BF16 = mybir.dt.bfloat16
I32 = mybir.dt.int32
AF = mybir.ActivationFunctionType
ALU = mybir.AluOpType
BF_NP = ml_dtypes.bfloat16

NCORES = 8
B = 128
BSH = B // NCORES  # 16 batch rows per core
D = 512
V = 1024
J = 2 * D  # 1024 decoder hidden
DC = D // 128  # 4 d-chunks
VT = V // 128  # 8 v-tiles
MAX_LEN = 5000


def build_nc(bd2_val: float) -> bass.Bass:
    # Bacc (not plain Bass): its finalize() legalizes sync waits
    # (generate_event_semaphores) to TRN2's 1-wait-per-instruction limit.
    nc = bacc.Bacc()

    # ---- per-core inputs ----
    te_d = nc.declare_dram_parameter("te", [BSH, D, V], BF16, isOutput=False)
    x_d = nc.declare_dram_parameter("x", [BSH, V], BF16, isOutput=False)
    ts_d = nc.declare_dram_parameter("ts", [BSH, 1], I32, isOutput=False)
    qet_d = nc.declare_dram_parameter("qet", [D, BSH], BF16, isOutput=False)
    # ---- replicated (host-folded) weights ----
    pe_d = nc.declare_dram_parameter("pe", [MAX_LEN, D], F32, isOutput=False)
    wt1t_d = nc.declare_dram_parameter("wt1t", [D, D], BF16, isOutput=False)
    wt2t_d = nc.declare_dram_parameter("wt2t", [D, D], BF16, isOutput=False)
    bt1c_d = nc.declare_dram_parameter("bt1c", [128, DC], F32, isOutput=False)
    bt2c_d = nc.declare_dram_parameter("bt2c", [128, DC], F32, isOutput=False)
    m1_d = nc.declare_dram_parameter("m1", [D, D], BF16, isOutput=False)
    m3_d = nc.declare_dram_parameter("m3", [D, J], BF16, isOutput=False)
    bm_d = nc.declare_dram_parameter("bm", [D, J], BF16, isOutput=False)
    r0_d = nc.declare_dram_parameter("r0", [J], BF16, isOutput=False)
    w2_d = nc.declare_dram_parameter("w2", [J], F32, isOutput=False)
    p_d = nc.declare_dram_parameter("p", [BSH, V], F32, isOutput=True)

    with tile.TileContext(nc) as tc:
        with (
            tc.tile_pool(name="w", bufs=1) as wp,
            tc.tile_pool(name="te", bufs=3) as tep,
            tc.tile_pool(name="scr", bufs=1) as scrp,
            tc.tile_pool(name="rows", bufs=3) as rowp,
            tc.tile_pool(name="sm", bufs=3) as smp,
            tc.tile_pool(name="tiny", bufs=6) as tinyp,
            tc.tile_pool(name="ebsb", bufs=2) as ebsb,
            tc.tile_pool(name="dramp", bufs=2, space="DRAM") as dramp,
            tc.tile_pool(name="hp", bufs=3, space="PSUM") as hp,
            tc.tile_pool(name="sp", bufs=2, space="PSUM") as sp,
        ):
            # ================= weight / constant loads =================
            wt1t = wp.tile([128, DC, D], BF16, tag="wt1t")
            nc.sync.dma_start(out=wt1t, in_=wt1t_d[:].rearrange("(c p) z -> p c z", p=128))
            wt2t = wp.tile([128, DC, D], BF16, tag="wt2t")
            nc.sync.dma_start(out=wt2t, in_=wt2t_d[:].rearrange("(c p) z -> p c z", p=128))
            m1 = wp.tile([128, DC, D], BF16, tag="m1")
            nc.sync.dma_start(out=m1, in_=m1_d[:].rearrange("(c p) z -> p c z", p=128))
            bt1c = wp.tile([128, DC], F32, tag="bt1c")
            nc.sync.dma_start(out=bt1c, in_=bt1c_d[:])
            bt2c = wp.tile([128, DC], F32, tag="bt2c")
            nc.sync.dma_start(out=bt2c, in_=bt2c_d[:])
            qet = wp.tile([128, DC, BSH], BF16, tag="qet")
            nc.sync.dma_start(out=qet, in_=qet_d[:].rearrange("(c p) b -> p c b", p=128))
            ts_sb = wp.tile([BSH, 1], I32, tag="ts")
            nc.sync.dma_start(out=ts_sb, in_=ts_d[:])
            # r0 staged on partition rows 0 and 32 (rhs of packed fold matmuls)
            r01 = wp.tile([33, J], BF16, tag="r01")
            for _r in (0, 32):
                nc.sync.dma_start(
                    out=r01[_r:_r + 1, :],
                    in_=bass.AP(tensor=r0_d, offset=0, ap=[[J, 1], [1, J]]),
                )
            ones_bf = wp.tile([1, 128], BF16, tag="ones_bf")
            nc.vector.memset(ones_bf, 1.0)
            # ones at partition rows 0 and 32 for row-packed K=1 fold matmuls
            ones4 = wp.tile([33, 128], BF16, tag="ones4")
            nc.vector.memset(ones4, 1.0)
            m3 = wp.tile([128, DC, J], BF16, tag="m3")
            nc.sync.dma_start(out=m3, in_=m3_d[:].rearrange("(c p) j -> p c j", p=128))
            bm = wp.tile([128, DC, J], BF16, tag="bm")
            nc.sync.dma_start(out=bm, in_=bm_d[:].rearrange("(c p) j -> p c j", p=128))
            # w2 replicated across partitions (fp32; only read by DVE)
            w2bc = wp.tile([128, J], F32, tag="w2bc")
            nc.sync.dma_start(
                out=w2bc,
                in_=bass.AP(tensor=w2_d, offset=0, ap=[[0, 128], [1, J]]),
            )
            id128 = wp.tile([128, 128], F32, tag="id128")
            from concourse.masks import make_identity

            make_identity(nc, id128)
            # bf16 identity produced by ACT (keeps transpose waits mergeable)
            id_bf = wp.tile([BSH, BSH], BF16, tag="id_bf")
            nc.scalar.activation(out=id_bf, in_=id128[:BSH, :BSH], func=AF.Copy)
            # PE warmup on id128 so later fp32 transposes never owe a Pool wait
            warm_ps = sp.tile([2, 2], F32, tag="ps")
            nc.tensor.transpose(warm_ps, id128[0:2, 0:2], id128[0:2, 0:2])

            # ================= setup: cond / q~ / CB =================
            # gather pe rows by timestep, cast to bf16
            tpe = wp.tile([BSH, D], F32, tag="tpe")
            nc.gpsimd.indirect_dma_start(
                out=tpe[:],
                out_offset=None,
                in_=pe_d[:],
                in_offset=bass.IndirectOffsetOnAxis(ap=ts_sb[:, :1], axis=0),
            )
            tpe_bf = wp.tile([BSH, D], BF16, tag="tpe_bf")
            nc.scalar.activation(out=tpe_bf, in_=tpe, func=AF.Copy)
            tpeT = wp.tile([128, DC, BSH], BF16, tag="tpeT")
            for c in range(DC):
                ps = sp.tile([128, BSH], BF16, tag="ps")
                nc.tensor.transpose(ps, tpe_bf[:, c * 128:(c + 1) * 128], id_bf)
                nc.scalar.activation(out=tpeT[:, c, :], in_=ps, func=AF.Copy)
            # Z.T = Wt1 @ tpe.T (+bt1), silu
            s_sb = wp.tile([128, DC, BSH], BF16, tag="s_sb")
            for zt in range(DC):
                ps = sp.tile([128, BSH], F32, tag="ps")
                for c in range(DC):
                    nc.tensor.matmul(
                        ps, wt1t[:, c, zt * 128:(zt + 1) * 128], tpeT[:, c, :],
                        start=(c == 0), stop=(c == DC - 1),
                    )
                nc.scalar.activation(
                    out=s_sb[:, zt, :], in_=ps, func=AF.Silu,
                    bias=bt1c[:, zt:zt + 1], scale=1.0,
                )
            # condT = Wt2 @ silu (+bt2)
            condT = wp.tile([128, DC, BSH], BF16, tag="condT")
            for ct in range(DC):
                ps = sp.tile([128, BSH], F32, tag="ps")
                for c in range(DC):
                    nc.tensor.matmul(
                        ps, wt2t[:, c, ct * 128:(ct + 1) * 128], s_sb[:, c, :],
                        start=(c == 0), stop=(c == DC - 1),
                    )
                nc.scalar.activation(
                    out=condT[:, ct, :], in_=ps, func=AF.Identity,
                    bias=bt2c[:, ct:ct + 1], scale=1.0,
                )
            # qcT = qeT + condT ; q~T = M1.T @ qcT, split into bf16 hi + lo
            qcT = wp.tile([128, DC, BSH], BF16, tag="qcT")
            nc.vector.tensor_add(qcT[:], qet[:], condT[:])
            qtT_hi = wp.tile([128, DC, BSH], BF16, tag="qtT_hi")
            qtT_lo = wp.tile([128, DC, BSH], BF16, tag="qtT_lo")
            for mt in range(DC):
                ps = sp.tile([128, BSH], F32, tag="ps")
                for c in range(DC):
                    nc.tensor.matmul(
                        ps, m1[:, c, mt * 128:(mt + 1) * 128], qcT[:, c, :],
                        start=(c == 0), stop=(c == DC - 1),
                    )
                nc.scalar.activation(out=qtT_hi[:, mt, :], in_=ps, func=AF.Copy)
                nc.vector.tensor_tensor(
                    out=qtT_lo[:, mt, :], in0=ps, in1=qtT_hi[:, mt, :],
                    op=ALU.subtract,
                )
            # ============ software-pipelined main loop over batch rows ============
            # Attention for row b+1 (scores/softmax/ws) is emitted before row
            # b's decoder so the PE never idles on the softmax chain and the
            # HAM clock stays warm. te/x rows are prefetched two rows ahead.
            st = [dict() for _ in range(BSH)]

            def emit_loads(b):
                s = st[b]
                s["xrow"] = rowp.tile([1, V], BF16, tag="xrow", name=f"xrow{b}")
                nc.sync.dma_start(out=s["xrow"], in_=x_d[b:b + 1, :])
                s["te"] = tep.tile([128, DC, V], BF16, tag="te", name=f"te{b}")
                nc.sync.dma_start(
                    out=s["te"], in_=te_d[b].rearrange("(c p) v -> p c v", p=128)
                )

            def emit_attention(b):
                """scores (PE) -> exp/norm (ACT/DVE) -> ebc (PE) -> ws (DVE)"""
                s = st[b]
                te_t, xrow = s["te"], s["xrow"]
                sc = []
                for h in range(2):
                    ps = sp.tile([1, 512], F32, tag="ps", name=f"sc{b}_{h}")
                    for c in range(DC):
                        nc.tensor.matmul(
                            ps, qtT_hi[:, c, b:b + 1],
                            te_t[:, c, h * 512:(h + 1) * 512],
                            start=(c == 0), stop=False,
                        )
                    sc.append(ps)
                for h in range(2):
                    nc.tensor.matmul(
                        sc[h], ones4[0:1, 0:1],
                        xrow[0:1, h * 512:(h + 1) * 512],
                        start=False, stop=True,
                    )
                exp_row = rowp.tile([1, V], F32, tag="exp", name=f"exp{b}")
                se = [tinyp.tile([1, 1], F32, tag="t1", name=f"se{h}_{b}") for h in range(2)]
                for h in range(2):
                    nc.scalar.activation(
                        out=exp_row[:, h * 512:(h + 1) * 512], in_=sc[h],
                        func=AF.Exp, accum_out=se[h],
                    )
                sume = tinyp.tile([1, 1], F32, tag="t1", name=f"sume{b}")
                nc.vector.tensor_add(sume, se[0], se[1])
                rec = tinyp.tile([1, 1], F32, tag="t1", name=f"rec{b}")
                nc.vector.reciprocal(rec, sume)
                expn = rowp.tile([1, V], BF16, tag="expn", name=f"expn{b}")
                nc.scalar.activation(
                    out=expn, in_=exp_row, func=AF.Copy, bias=0.0, scale=rec[:, :1]
                )
                s["expn"] = expn
                # weight row replicated across partitions: bounce through
                # DRAM (DRAM->SBUF partition-broadcast DMA is legal; frees two
                # PSUM banks for a third H tile and removes two PE matmuls)
                ebounce = dramp.tile([1, V], BF16, tag="eb", name=f"eb{b}")
                nc.sync.dma_start(out=ebounce, in_=expn)
                ebcs = ebsb.tile([128, V], BF16, tag="ebcs", name=f"ebcs{b}")
                nc.sync.dma_start(
                    out=ebcs,
                    in_=bass.AP(tensor=ebounce.tensor, offset=ebounce.offset,
                                ap=[[0, 128]] + ebounce.ap[1:]),
                )
                s["ebc"] = ebcs

            def emit_ws(b):
                """wsum columns via DVE reduce over v; emitted after the
                previous row's epilogue so the DVE pace matches the PE."""
                s = st[b]
                te_t, ebc = s["te"], s["ebc"]
                ws2 = tinyp.tile([128, DC], F32, tag="ws2", name=f"ws2_{b}")
                ws_sb = smp.tile([128, DC], BF16, tag="ws", name=f"ws{b}")
                wscr = scrp.tile([128, V], BF16, tag="wscr")
                for c in range(DC):
                    nc.vector.scalar_tensor_tensor(
                        out=wscr, in0=te_t[:, c, :], scalar=0.0, in1=ebc,
                        op0=ALU.bypass, op1=ALU.mult,
                        accum_out=ws2[:, c:c + 1],
                    )
                for c in range(DC):
                    nc.vector.scalar_tensor_tensor(
                        out=ws_sb[:, c:c + 1], in0=ws2[:, c:c + 1],
                        scalar=condT[:, c, b:b + 1], in1=zero1,
                        op0=ALU.add, op1=ALU.add,
                    )
                s["ws"] = ws_sb

            def emit_late_attention(b):
                """expT transposes + base matvec (PE) + base copies (ACT)"""
                s = st[b]
                expn, ws_sb = s["expn"], s["ws"]
                expT_ps = sp.tile([128, VT, 2], BF16, tag="ps", name=f"expTp{b}")
                for vt in range(VT):
                    nc.tensor.transpose(
                        expT_ps[:, vt, 0:1],
                        expn[:, vt * 128:(vt + 1) * 128],
                        ones_bf[0:1, 0:1],
                    )
                expT = smp.tile([128, VT], F32, tag="expT", name=f"expT{b}")
                nc.vector.tensor_copy(expT, expT_ps[:, :, 0])
                s["expT"] = expT
                base4 = smp.tile([33, J], BF16, tag="base", name=f"base{b}")
                bps = []
                for h in range(2):
                    ps = sp.tile([1, 512], F32, tag="ps", name=f"basep{b}_{h}")
                    for c in range(DC):
                        nc.tensor.matmul(
                            ps, ws_sb[:, c:c + 1], m3[:, c, h * 512:(h + 1) * 512],
                            start=(c == 0), stop=False,
                        )
                    bps.append(ps)
                for h in range(2):
                    nc.tensor.matmul(
                        bps[h], ones4[0:1, 0:1], r01[0:1, h * 512:(h + 1) * 512],
                        start=False, stop=True,
                    )
                for h in range(2):
                    nc.scalar.activation(
                        out=base4[0:1, h * 512:(h + 1) * 512], in_=bps[h], func=AF.Copy
                    )
                bbounce = dramp.tile([1, J], BF16, tag="bb", name=f"bb{b}")
                nc.sync.dma_start(out=bbounce, in_=base4[0:1, :])
                base_bc = ebsb.tile([128, J], BF16, tag="bbc", name=f"bbc{b}")
                nc.sync.dma_start(
                    out=base_bc,
                    in_=bass.AP(tensor=bbounce.tensor, offset=bbounce.offset,
                                ap=[[0, 128]] + bbounce.ap[1:]),
                )
                s["base"] = base_bc

            def emit_decoder(b):
                """H matmuls + base fold (PE) + fused relu*w2 rowsum (DVE)"""
                s = st[b]
                te_t, base_sb, expT = s["te"], s["base"], s["expT"]
                ht = [None] * VT
                H_PREFILL = 2

                def emit_h(vt):
                    t = hp.tile([128, 2, 512], F32, tag="h", name=f"h{b}_{vt}")
                    for c in range(DC):
                        for h in range(2):
                            nc.tensor.matmul(
                                t[:, h, :],
                                te_t[:, c, vt * 128:(vt + 1) * 128],
                                bm[:, c, h * 512:(h + 1) * 512],
                                start=(c == 0), stop=False,
                            )
                    ht[vt] = t

                for vt in range(H_PREFILL):
                    emit_h(vt)
                p_cols = smp.tile([128, VT], F32, tag="pcols", name=f"pc{b}")
                for vt in range(VT):
                    t = ht[vt]
                    for h in range(2):
                        r = 32 * h
                        nc.tensor.matmul(
                            t[:, h, :], ones4[r:r + 1, :],
                            base_sb[r:r + 1, h * 512:(h + 1) * 512],
                            start=False, stop=True, tile_position=(r, 0),
                        )
                    t_flat = t.rearrange("p a v -> p (a v)")
                    pacc = tinyp.tile([128, 1], F32, tag="pacc", name=f"pacc{b}_{vt}")
                    nc.vector.scalar_tensor_tensor(
                        out=t_flat, in0=t_flat, scalar=0.0, in1=w2bc,
                        op0=ALU.max, op1=ALU.mult, accum_out=pacc,
                    )
                    nc.vector.scalar_tensor_tensor(
                        out=p_cols[:, vt:vt + 1], in0=pacc, scalar=bd2_val,
                        in1=expT[:, vt:vt + 1], op0=ALU.add, op1=ALU.add,
                    )
                    if vt + H_PREFILL < VT:
                        emit_h(vt + H_PREFILL)
                # transpose p columns -> row chunks, DMA out
                ptr = sp.tile([VT, 128], F32, tag="ps", name=f"ptr{b}")
                nc.tensor.transpose(ptr, p_cols, id128)
                p_row = smp.tile([VT, 128], F32, tag="prow", name=f"prow{b}")
                nc.scalar.copy(out=p_row, in_=ptr)
                nc.sync.dma_start(
                    out=p_d[b].rearrange("(t p) -> t p", p=128), in_=p_row
                )

            zero1 = wp.tile([128, 1], F32, tag="zero1")
            nc.vector.memset(zero1, 0.0)
            emit_loads(0)
            emit_loads(1)
            emit_attention(0)
            emit_ws(0)
            for b in range(BSH):
                if b + 2 < BSH:
                    emit_loads(b + 2)
                if b + 1 < BSH:
                    emit_attention(b + 1)
                emit_late_attention(b)
                emit_decoder(b)
                if b + 1 < BSH:
                    emit_ws(b + 1)
                st[b].clear()

    return nc


_NC_CACHE: dict = {}


def _get_nc(bd2_val: float) -> bass.Bass:
    key = float(bd2_val)
    if key not in _NC_CACHE:
        nc = build_nc(key)
        nc.finalize()
        _NC_CACHE[key] = nc
    return _NC_CACHE[key]


def _pos_encoding() -> np.ndarray:
    pos = np.arange(MAX_LEN, dtype=np.float32)[:, None]
    div = np.exp(np.arange(0, D, 2, dtype=np.float32) * (-np.log(10000.0) / D))
    pe = np.zeros((MAX_LEN, D), dtype=np.float32)
    pe[:, 0::2] = np.sin(pos * div)
    pe[:, 1::2] = np.cos(pos * div)
    return pe


def prepare_in_maps(inputs: dict) -> tuple[list, float]:
    f32 = lambda a: np.ascontiguousarray(np.asarray(a), dtype=np.float32)
    bf = lambda a: np.ascontiguousarray(np.asarray(a, dtype=np.float32).astype(BF_NP))
    x = np.asarray(inputs["x"], dtype=np.float32)
    ts = np.ascontiguousarray(np.asarray(inputs["timesteps"]).astype(np.int32).reshape(B, 1))
    qe = np.asarray(inputs["query_emb"], dtype=np.float32)
    te = np.asarray(inputs["target_emb"], dtype=np.float32)
    Wq, Wk, Wv, Wp = (f32(inputs[k]) for k in ("Wq", "Wk", "Wv", "Wp"))
    bp = f32(inputs["bp"])
    Wt1, bt1, Wt2, bt2 = (f32(inputs[k]) for k in ("Wt1", "bt1", "Wt2", "bt2"))
    Wd1, bd1, Wd2, bd2 = (f32(inputs[k]) for k in ("Wd1", "bd1", "Wd2", "bd2"))

    pe = _pos_encoding()
    M1 = Wq.T @ Wk
    A = np.ascontiguousarray(Wd1[:, :D].T)
    Bm = Wd1[:, D:].T
    M3 = (Wv.T @ Wp.T) @ A
    r0 = np.ascontiguousarray(bp @ A + bd1)
    w2 = np.ascontiguousarray(Wd2[0])
    bd2_val = float(bd2.reshape(-1)[0])
    bt1c = np.ascontiguousarray(bt1.reshape(DC, 128).T)
    bt2c = np.ascontiguousarray(bt2.reshape(DC, 128).T)

    shared = dict(
        pe=pe, wt1t=bf(Wt1.T), wt2t=bf(Wt2.T), bt1c=bt1c, bt2c=bt2c,
        m1=bf(M1), m3=bf(M3), bm=bf(Bm), r0=bf(r0), w2=w2,
    )
    in_maps = []
    for i in range(NCORES):
        s = slice(i * BSH, (i + 1) * BSH)
        in_maps.append(
            dict(
                te=bf(te[s].transpose(0, 2, 1)),
                x=bf(x[s]),
                ts=np.ascontiguousarray(ts[s]),
                qet=bf(qe[s].T),
                **shared,
            )
        )
    return in_maps, bd2_val


def run(inputs: dict, trace: bool = False):
    in_maps, bd2_val = prepare_in_maps(inputs)
    nc = _get_nc(bd2_val)
    res = run_bass_kernel_spmd(nc, in_maps, list(range(NCORES)), trace=trace)
    out = np.concatenate([r["p"] for r in res.results], axis=0).astype(np.float32)
    return out, res


def kernel(**inputs) -> np.ndarray:
    out, _ = run(inputs, trace=False)
    return out


# revision 65
# speedup vs baseline: 1.2708x; 1.2708x over previous
"""Trainium2 Bass kernel for the single-query-attention diffusion decoder.

Full-input contract: kernel(**inputs) -> np.ndarray [B, V].
Data-parallel over batch across 8 NeuronCores (16 rows each).

Math (reference restructured):
    cond  = silu(pe[t] @ Wt1.T + bt1) @ Wt2.T + bt2            [B, D]
    q~    = (query + cond) @ M1,  M1 = Wq.T @ Wk               [B, D]
    s[v]  = q~ . T[v] + x[v]   (+ q~.cond, dropped: softmax shift-invariant)
    w     = softmax(s)
    wsum  = sum_v w[v] T[v] + cond                             [D]
    base  = wsum @ M3 + cond @ Bm + r0                         [2D]
            M3 = Wv.T @ Wp.T @ Wd1[:, :D].T,  Bm = Wd1[:, D:].T
            r0 = bp @ Wd1[:, :D].T + bd1
    p[v]  = sum_j relu(T[v] @ Bm + base)[j] * w2[j] + bd2 + w[v]

The target shard is passed host-transposed [16, D, V] in bf16 so the PE
contraction dim (d) lands on SBUF partitions with no on-device transposes
and half the HBM traffic. Matmuls run in bf16 (PSUM accumulates fp32);
q~ uses a bf16 hi+lo split to keep softmax logits near-fp32 accurate; the
decoder relu/w2 reduction and all final adds stay in fp32.
"""

import os
import sys

for _p in ("/opt/trn_rl_repo", "/opt/trn_rl_repo/concourse"):
    if os.path.isdir(_p) and _p not in sys.path:
        sys.path.append(_p)

import numpy as np
import ml_dtypes

import concourse.bass as bass
import concourse.tile as tile
from concourse import bacc, mybir
from concourse.bass_utils import run_bass_kernel_spmd

F32 = mybir.dt.float32
BF16 = mybir.dt.bfloat16
I32 = mybir.dt.int32
AF = mybir.ActivationFunctionType
ALU = mybir.AluOpType
BF_NP = ml_dtypes.bfloat16

NCORES = 8
B = 128
BSH = B // NCORES  # 16 batch rows per core
D = 512
V = 1024
J = 2 * D  # 1024 decoder hidden
DC = D // 128  # 4 d-chunks
VT = V // 128  # 8 v-tiles
MAX_LEN = 5000


def build_nc(bd2_val: float) -> bass.Bass:
    # Bacc (not plain Bass): its finalize() legalizes sync waits
    # (generate_event_semaphores) to TRN2's 1-wait-per-instruction limit.
    nc = bacc.Bacc()

    # ---- per-core inputs ----
    te_d = nc.declare_dram_parameter("te", [BSH, D, V], BF16, isOutput=False)
    x_d = nc.declare_dram_parameter("x", [BSH, V], BF16, isOutput=False)
    ts_d = nc.declare_dram_parameter("ts", [BSH, 1], I32, isOutput=False)
    qet_d = nc.declare_dram_parameter("qet", [D, BSH], BF16, isOutput=False)
    # ---- replicated (host-folded) weights ----
    pe_d = nc.declare_dram_parameter("pe", [MAX_LEN, D], F32, isOutput=False)
    wt1t_d = nc.declare_dram_parameter("wt1t", [D, D], BF16, isOutput=False)
    wt2t_d = nc.declare_dram_parameter("wt2t", [D, D], BF16, isOutput=False)
    bt1c_d = nc.declare_dram_parameter("bt1c", [128, DC], F32, isOutput=False)
    bt2c_d = nc.declare_dram_parameter("bt2c", [128, DC], F32, isOutput=False)
    m1_d = nc.declare_dram_parameter("m1", [D, D], BF16, isOutput=False)
    m3_d = nc.declare_dram_parameter("m3", [D, J], BF16, isOutput=False)
    bm_d = nc.declare_dram_parameter("bm", [D, J], BF16, isOutput=False)
    r0_d = nc.declare_dram_parameter("r0", [J], BF16, isOutput=False)
    w2_d = nc.declare_dram_parameter("w2", [J], F32, isOutput=False)
    p_d = nc.declare_dram_parameter("p", [BSH, V], F32, isOutput=True)

    with tile.TileContext(nc) as tc:
        with (
            tc.tile_pool(name="w", bufs=1) as wp,
            tc.tile_pool(name="te", bufs=3) as tep,
            tc.tile_pool(name="scr", bufs=1) as scrp,
            tc.tile_pool(name="rows", bufs=3) as rowp,
            tc.tile_pool(name="sm", bufs=3) as smp,
            tc.tile_pool(name="tiny", bufs=6) as tinyp,
            tc.tile_pool(name="ebsb", bufs=2) as ebsb,
            tc.tile_pool(name="dramp", bufs=2, space="DRAM") as dramp,
            tc.tile_pool(name="hp", bufs=3, space="PSUM") as hp,
            tc.tile_pool(name="sp", bufs=2, space="PSUM") as sp,
        ):
            # ================= weight / constant loads =================
            wt1t = wp.tile([128, DC, D], BF16, tag="wt1t")
            nc.sync.dma_start(out=wt1t, in_=wt1t_d[:].rearrange("(c p) z -> p c z", p=128))
            wt2t = wp.tile([128, DC, D], BF16, tag="wt2t")
            nc.sync.dma_start(out=wt2t, in_=wt2t_d[:].rearrange("(c p) z -> p c z", p=128))
            m1 = wp.tile([128, DC, D], BF16, tag="m1")
            nc.sync.dma_start(out=m1, in_=m1_d[:].rearrange("(c p) z -> p c z", p=128))
            bt1c = wp.tile([128, DC], F32, tag="bt1c")
            nc.sync.dma_start(out=bt1c, in_=bt1c_d[:])
            bt2c = wp.tile([128, DC], F32, tag="bt2c")
            nc.sync.dma_start(out=bt2c, in_=bt2c_d[:])
            qet = wp.tile([128, DC, BSH], BF16, tag="qet")
            nc.sync.dma_start(out=qet, in_=qet_d[:].rearrange("(c p) b -> p c b", p=128))
            ts_sb = wp.tile([BSH, 1], I32, tag="ts")
            nc.sync.dma_start(out=ts_sb, in_=ts_d[:])
            # r0 staged on partition rows 0 and 32 (rhs of packed fold matmuls)
            r01 = wp.tile([33, J], BF16, tag="r01")
            for _r in (0, 32):
                nc.sync.dma_start(
                    out=r01[_r:_r + 1, :],
                    in_=bass.AP(tensor=r0_d, offset=0, ap=[[J, 1], [1, J]]),
                )
            ones_bf = wp.tile([1, 128], BF16, tag="ones_bf")
            nc.vector.memset(ones_bf, 1.0)
            # ones at partition rows 0 and 32 for row-packed K=1 fold matmuls
            ones4 = wp.tile([33, 128], BF16, tag="ones4")
            nc.vector.memset(ones4, 1.0)
            m3 = wp.tile([128, DC, J], BF16, tag="m3")
            nc.sync.dma_start(out=m3, in_=m3_d[:].rearrange("(c p) j -> p c j", p=128))
            bm = wp.tile([128, DC, J], BF16, tag="bm")
            nc.sync.dma_start(out=bm, in_=bm_d[:].rearrange("(c p) j -> p c j", p=128))
            # w2 replicated across partitions (fp32; only read by DVE)
            w2bc = wp.tile([128, J], F32, tag="w2bc")
            nc.sync.dma_start(
                out=w2bc,
                in_=bass.AP(tensor=w2_d, offset=0, ap=[[0, 128], [1, J]]),
            )
            id128 = wp.tile([128, 128], F32, tag="id128")
            from concourse.masks import make_identity

            make_identity(nc, id128)
            # bf16 identity produced by ACT (keeps transpose waits mergeable)
            id_bf = wp.tile([BSH, BSH], BF16, tag="id_bf")
            nc.scalar.activation(out=id_bf, in_=id128[:BSH, :BSH], func=AF.Copy)
            # PE warmup on id128 so later fp32 transposes never owe a Pool wait
            warm_ps = sp.tile([2, 2], F32, tag="ps")
            nc.tensor.transpose(warm_ps, id128[0:2, 0:2], id128[0:2, 0:2])

            # ================= setup: cond / q~ / CB =================
            # gather pe rows by timestep, cast to bf16
            tpe = wp.tile([BSH, D], F32, tag="tpe")
            nc.gpsimd.indirect_dma_start(
                out=tpe[:],
                out_offset=None,
                in_=pe_d[:],
                in_offset=bass.IndirectOffsetOnAxis(ap=ts_sb[:, :1], axis=0),
            )
            tpe_bf = wp.tile([BSH, D], BF16, tag="tpe_bf")
            nc.scalar.activation(out=tpe_bf, in_=tpe, func=AF.Copy)
            tpeT = wp.tile([128, DC, BSH], BF16, tag="tpeT")
            for c in range(DC):
                ps = sp.tile([128, BSH], BF16, tag="ps")
                nc.tensor.transpose(ps, tpe_bf[:, c * 128:(c + 1) * 128], id_bf)
                nc.scalar.activation(out=tpeT[:, c, :], in_=ps, func=AF.Copy)
            # Z.T = Wt1 @ tpe.T (+bt1), silu
            s_sb = wp.tile([128, DC, BSH], BF16, tag="s_sb")
            for zt in range(DC):
                ps = sp.tile([128, BSH], F32, tag="ps")
                for c in range(DC):
                    nc.tensor.matmul(
                        ps, wt1t[:, c, zt * 128:(zt + 1) * 128], tpeT[:, c, :],
                        start=(c == 0), stop=(c == DC - 1),
                    )
                nc.scalar.activation(
                    out=s_sb[:, zt, :], in_=ps, func=AF.Silu,
                    bias=bt1c[:, zt:zt + 1], scale=1.0,
                )
            # condT = Wt2 @ silu (+bt2)
            condT = wp.tile([128, DC, BSH], BF16, tag="condT")
            for ct in range(DC):
                ps = sp.tile([128, BSH], F32, tag="ps")
                for c in range(DC):
                    nc.tensor.matmul(
                        ps, wt2t[:, c, ct * 128:(ct + 1) * 128], s_sb[:, c, :],
                        start=(c == 0), stop=(c == DC - 1),
                    )
                nc.scalar.activation(
                    out=condT[:, ct, :], in_=ps, func=AF.Identity,
                    bias=bt2c[:, ct:ct + 1], scale=1.0,
                )
            # qcT = qeT + condT ; q~T = M1.T @ qcT, split into bf16 hi + lo
            qcT = wp.tile([128, DC, BSH], BF16, tag="qcT")
            nc.vector.tensor_add(qcT[:], qet[:], condT[:])
            qtT_hi = wp.tile([128, DC, BSH], BF16, tag="qtT_hi")
            qtT_lo = wp.tile([128, DC, BSH], BF16, tag="qtT_lo")
            for mt in range(DC):
                ps = sp.tile([128, BSH], F32, tag="ps")
                for c in range(DC):
                    nc.tensor.matmul(
                        ps, m1[:, c, mt * 128:(mt + 1) * 128], qcT[:, c, :],
                        start=(c == 0), stop=(c == DC - 1),
                    )
                nc.scalar.activation(out=qtT_hi[:, mt, :], in_=ps, func=AF.Copy)
                nc.vector.tensor_tensor(
                    out=qtT_lo[:, mt, :], in0=ps, in1=qtT_hi[:, mt, :],
                    op=ALU.subtract,
                )
            # ============ software-pipelined main loop over batch rows ============
            # Attention for row b+1 (scores/softmax/ws) is emitted before row
            # b's decoder so the PE never idles on the softmax chain and the
            # HAM clock stays warm. te/x rows are prefetched two rows ahead.
            st = [dict() for _ in range(BSH)]

            def emit_loads(b):
                s = st[b]
                s["xrow"] = rowp.tile([1, V], BF16, tag="xrow", name=f"xrow{b}")
                nc.sync.dma_start(out=s["xrow"], in_=x_d[b:b + 1, :])
                s["te"] = tep.tile([128, DC, V], BF16, tag="te", name=f"te{b}")
                nc.sync.dma_start(
                    out=s["te"], in_=te_d[b].rearrange("(c p) v -> p c v", p=128)
                )

            def emit_attention(b):
                """scores (PE) -> exp/norm (ACT/DVE) -> ebc (PE) -> ws (DVE)"""
                s = st[b]
                te_t, xrow = s["te"], s["xrow"]
                sc = []
                for h in range(2):
                    ps = sp.tile([1, 512], F32, tag="ps", name=f"sc{b}_{h}")
                    for c in range(DC):
                        nc.tensor.matmul(
                            ps, qtT_hi[:, c, b:b + 1],
                            te_t[:, c, h * 512:(h + 1) * 512],
                            start=(c == 0), stop=False,
                        )
                    sc.append(ps)
                for h in range(2):
                    nc.tensor.matmul(
                        sc[h], ones4[0:1, 0:1],
                        xrow[0:1, h * 512:(h + 1) * 512],
                        start=False, stop=True,
                    )
                exp_row = rowp.tile([1, V], F32, tag="exp", name=f"exp{b}")
                se = [tinyp.tile([1, 1], F32, tag="t1", name=f"se{h}_{b}") for h in range(2)]
                for h in range(2):
                    nc.scalar.activation(
                        out=exp_row[:, h * 512:(h + 1) * 512], in_=sc[h],
                        func=AF.Exp, accum_out=se[h],
                    )
                sume = tinyp.tile([1, 1], F32, tag="t1", name=f"sume{b}")
                nc.vector.tensor_add(sume, se[0], se[1])
                rec = tinyp.tile([1, 1], F32, tag="t1", name=f"rec{b}")
                nc.vector.reciprocal(rec, sume)
                expn = rowp.tile([1, V], BF16, tag="expn", name=f"expn{b}")
                nc.scalar.activation(
                    out=expn, in_=exp_row, func=AF.Copy, bias=0.0, scale=rec[:, :1]
                )
                s["expn"] = expn
                # weight row replicated across partitions: bounce through
                # DRAM (DRAM->SBUF partition-broadcast DMA is legal; frees two
                # PSUM banks for a third H tile and removes two PE matmuls)
                ebounce = dramp.tile([1, V], BF16, tag="eb", name=f"eb{b}")
                nc.sync.dma_start(out=ebounce, in_=expn)
                ebcs = ebsb.tile([128, V], BF16, tag="ebcs", name=f"ebcs{b}")
                nc.sync.dma_start(
                    out=ebcs,
                    in_=bass.AP(tensor=ebounce.tensor, offset=ebounce.offset,
                                ap=[[0, 128]] + ebounce.ap[1:]),
                )
                s["ebc"] = ebcs

            def emit_ws(b):
                """wsum columns via DVE reduce over v; emitted after the
                previous row's epilogue so the DVE pace matches the PE."""
                s = st[b]
                te_t, ebc = s["te"], s["ebc"]
                ws2 = tinyp.tile([128, DC], F32, tag="ws2", name=f"ws2_{b}")
                ws_sb = smp.tile([128, DC], BF16, tag="ws", name=f"ws{b}")
                wscr = scrp.tile([128, V], BF16, tag="wscr")
                for c in range(DC):
                    nc.vector.scalar_tensor_tensor(
                        out=wscr, in0=te_t[:, c, :], scalar=0.0, in1=ebc,
                        op0=ALU.bypass, op1=ALU.mult,
                        accum_out=ws2[:, c:c + 1],
                    )
                for c in range(DC):
                    nc.vector.scalar_tensor_tensor(
                        out=ws_sb[:, c:c + 1], in0=ws2[:, c:c + 1],
                        scalar=condT[:, c, b:b + 1], in1=zero1,
                        op0=ALU.add, op1=ALU.add,
                    )
                s["ws"] = ws_sb

            def emit_late_attention(b):
                """expT transposes + base matvec (PE) + base copies (ACT)"""
                s = st[b]
                expn, ws_sb = s["expn"], s["ws"]
                expT_ps = sp.tile([128, VT, 2], BF16, tag="ps", name=f"expTp{b}")
                for vt in range(VT):
                    nc.tensor.transpose(
                        expT_ps[:, vt, 0:1],
                        expn[:, vt * 128:(vt + 1) * 128],
                        ones_bf[0:1, 0:1],
                    )
                expT = smp.tile([128, VT], F32, tag="expT", name=f"expT{b}")
                nc.vector.tensor_copy(expT, expT_ps[:, :, 0])
                s["expT"] = expT
                base4 = smp.tile([33, J], BF16, tag="base", name=f"base{b}")
                bps = []
                for h in range(2):
                    ps = sp.tile([1, 512], F32, tag="ps", name=f"basep{b}_{h}")
                    for c in range(DC):
                        nc.tensor.matmul(
                            ps, ws_sb[:, c:c + 1], m3[:, c, h * 512:(h + 1) * 512],
                            start=(c == 0), stop=False,
                        )
                    bps.append(ps)
                for h in range(2):
                    nc.tensor.matmul(
                        bps[h], ones4[0:1, 0:1], r01[0:1, h * 512:(h + 1) * 512],
                        start=False, stop=True,
                    )
                for h in range(2):
                    nc.scalar.activation(
                        out=base4[0:1, h * 512:(h + 1) * 512], in_=bps[h], func=AF.Copy
                    )
                s["base"] = base4

            def emit_decoder(b):
                """H matmuls + base fold (PE) + fused relu*w2 rowsum (DVE)"""
                s = st[b]
                te_t, base_sb, expT = s["te"], s["base"], s["expT"]
                ht = [None] * VT
                H_PREFILL = 2

                def emit_h(vt):
                    t = hp.tile([128, 2, 512], F32, tag="h", name=f"h{b}_{vt}")
                    for c in range(DC):
                        for h in range(2):
                            nc.tensor.matmul(
                                t[:, h, :],
                                te_t[:, c, vt * 128:(vt + 1) * 128],
                                bm[:, c, h * 512:(h + 1) * 512],
                                start=(c == 0), stop=False,
                            )
                    ht[vt] = t

                for vt in range(H_PREFILL):
                    emit_h(vt)
                p_cols = smp.tile([128, VT], F32, tag="pcols", name=f"pc{b}")
                for vt in range(VT):
                    t = ht[vt]
                    for h in range(2):
                        nc.tensor.matmul(
                            t[:, h, :], ones4[0:1, :],
                            base_sb[0:1, h * 512:(h + 1) * 512],
                            start=False, stop=True,
                        )
                    t_flat = t.rearrange("p a v -> p (a v)")
                    pacc = tinyp.tile([128, 1], F32, tag="pacc", name=f"pacc{b}_{vt}")
                    nc.vector.scalar_tensor_tensor(
                        out=t_flat, in0=t_flat, scalar=0.0, in1=w2bc,
                        op0=ALU.max, op1=ALU.mult, accum_out=pacc,
                    )
                    nc.vector.scalar_tensor_tensor(
                        out=p_cols[:, vt:vt + 1], in0=pacc, scalar=bd2_val,
                        in1=expT[:, vt:vt + 1], op0=ALU.add, op1=ALU.add,
                    )
                    if vt + H_PREFILL < VT:
                        emit_h(vt + H_PREFILL)
                # transpose p columns -> row chunks, DMA out
                ptr = sp.tile([VT, 128], F32, tag="ps", name=f"ptr{b}")
                nc.tensor.transpose(ptr, p_cols, id128)
                p_row = smp.tile([VT, 128], F32, tag="prow", name=f"prow{b}")
                nc.scalar.copy(out=p_row, in_=ptr)
                nc.sync.dma_start(
                    out=p_d[b].rearrange("(t p) -> t p", p=128), in_=p_row
                )

            zero1 = wp.tile([128, 1], F32, tag="zero1")
            nc.vector.memset(zero1, 0.0)
            emit_loads(0)
            emit_loads(1)
            emit_attention(0)
            emit_ws(0)
            for b in range(BSH):
                if b + 2 < BSH:
                    emit_loads(b + 2)
                if b + 1 < BSH:
                    emit_attention(b + 1)
                emit_late_attention(b)
                emit_decoder(b)
                if b + 1 < BSH:
                    emit_ws(b + 1)
                st[b].clear()

    return nc


_NC_CACHE: dict = {}


def _get_nc(bd2_val: float) -> bass.Bass:
    key = float(bd2_val)
    if key not in _NC_CACHE:
        nc = build_nc(key)
        nc.finalize()
        _NC_CACHE[key] = nc
    return _NC_CACHE[key]


def _pos_encoding() -> np.ndarray:
    pos = np.arange(MAX_LEN, dtype=np.float32)[:, None]
    div = np.exp(np.arange(0, D, 2, dtype=np.float32) * (-np.log(10000.0) / D))
    pe = np.zeros((MAX_LEN, D), dtype=np.float32)
    pe[:, 0::2] = np.sin(pos * div)
    pe[:, 1::2] = np.cos(pos * div)
    return pe


def prepare_in_maps(inputs: dict) -> tuple[list, float]:
    f32 = lambda a: np.ascontiguousarray(np.asarray(a), dtype=np.float32)
    bf = lambda a: np.ascontiguousarray(np.asarray(a, dtype=np.float32).astype(BF_NP))
    x = np.asarray(inputs["x"], dtype=np.float32)
    ts = np.ascontiguousarray(np.asarray(inputs["timesteps"]).astype(np.int32).reshape(B, 1))
    qe = np.asarray(inputs["query_emb"], dtype=np.float32)
    te = np.asarray(inputs["target_emb"], dtype=np.float32)
    Wq, Wk, Wv, Wp = (f32(inputs[k]) for k in ("Wq", "Wk", "Wv", "Wp"))
    bp = f32(inputs["bp"])
    Wt1, bt1, Wt2, bt2 = (f32(inputs[k]) for k in ("Wt1", "bt1", "Wt2", "bt2"))
    Wd1, bd1, Wd2, bd2 = (f32(inputs[k]) for k in ("Wd1", "bd1", "Wd2", "bd2"))

    pe = _pos_encoding()
    M1 = Wq.T @ Wk
    A = np.ascontiguousarray(Wd1[:, :D].T)
    Bm = Wd1[:, D:].T
    M3 = (Wv.T @ Wp.T) @ A
    r0 = np.ascontiguousarray(bp @ A + bd1)
    w2 = np.ascontiguousarray(Wd2[0])
    bd2_val = float(bd2.reshape(-1)[0])
    bt1c = np.ascontiguousarray(bt1.reshape(DC, 128).T)
    bt2c = np.ascontiguousarray(bt2.reshape(DC, 128).T)

    shared = dict(
        pe=pe, wt1t=bf(Wt1.T), wt2t=bf(Wt2.T), bt1c=bt1c, bt2c=bt2c,
        m1=bf(M1), m3=bf(M3), bm=bf(Bm), r0=bf(r0), w2=w2,
    )
    in_maps = []
    for i in range(NCORES):
        s = slice(i * BSH, (i + 1) * BSH)
        in_maps.append(
            dict(
                te=bf(te[s].transpose(0, 2, 1)),
                x=bf(x[s]),
                ts=np.ascontiguousarray(ts[s]),
                qet=bf(qe[s].T),
                **shared,
            )
        )
    return in_maps, bd2_val


def run(inputs: dict, trace: bool = False):
    in_maps, bd2_val = prepare_in_maps(inputs)
    nc = _get_nc(bd2_val)
    res = run_bass_kernel_spmd(nc, in_maps, list(range(NCORES)), trace=trace)
    out = np.concatenate([r["p"] for r in res.results], axis=0).astype(np.float32)
    return out, res


def kernel(**inputs) -> np.ndarray:
    out, _ = run(inputs, trace=False)
    return out
